# revision 23
# baseline (speedup 1.0000x reference)
"""Trainium2 Bass kernel for the conditional-prior VAE rational-quadratic
spline loss head.

Math (per row n of out_params [N, 48], eps [N, 2], nb = 8):
  y_loc = out_params[:, :2]
  raw_w = out_params[:, 2:18]  -> per spline-subrow j: channels 2+8j .. 9+8j
  raw_h = out_params[:, 18:34] -> channels 18+8j .. 25+8j
  raw_d = out_params[:, 34:48] -> channels 34+7j .. 40+7j
  y[n, j] = y_loc[n, j] + RQspline(eps[n, j]; raw_w, raw_h, raw_d)

Kernel strategy: pure data parallel over 8 cores (batch split). Per core,
rows are laid out 128 per partition-dim x T per free-dim. All cross-bin
work (softmax, cumsum, searchsorted, gather) is done as elementwise ops
over per-bin channel slices of row-major tiles:
  - softmax/cumsum in exp-space; searchsorted runs unnormalized
    (SW*C_k + MINB*k*S <= S*(x+3.5)) so no reciprocal sits on the
    critical path; SW/S for the final affine maps comes from one ACT
    exp(-ln S + ln SW).
  - gather at bin = masked sums  sum_k ge_k * v_k  folded by a small
    adjacency tree
  - the rational quadratic is evaluated in a division-free form (numer
    and denom both scaled by (w+eps)^3) so a single reciprocal remains.
Engine split is by parallel subgraph: DVE owns cumsum/searchsorted/W-path/
stage-C/final; GpSimd (which only supports plain tensor_tensor/copy ops)
owns derivative deltas, H/D-path products, the first tree fold and the
denominator branch; ACT owns all transcendentals (one act-table).
"""

import sys

for _p in ("/opt/trn_rl_repo",):
    if _p not in sys.path:
        sys.path.insert(0, _p)

import math
from contextlib import ExitStack

import numpy as np

import concourse.bass as bass
import concourse.tile as tile
from concourse import bacc, mybir
from concourse.bass_utils import run_bass_kernel_spmd

# Steer every activation to the one table set that holds all functions this
# kernel uses (exp, ln, abs, square, copy, identity), so the schedule carries
# a single LoadActFuncSet instead of reloading on every exp<->ln switch.
# Index order of the table list is preserved (walrus remaps by index).
_AF = mybir.ActivationFunctionType
_KERNEL_FUNCS = {_AF.Exp, _AF.Ln, _AF.Abs, _AF.Square, _AF.Copy, _AF.Identity}
_ONE_TABLE = "natural_log_exp_and_others"
_orig_get_act_tables = bacc.get_activation_tables


def _patched_act_tables(arch):
    out = {}
    for name, funcs in _orig_get_act_tables(arch).items():
        if name == _ONE_TABLE:
            out[name] = funcs
        else:
            out[name] = funcs - _KERNEL_FUNCS
    return out


bacc.get_activation_tables = _patched_act_tables

F32 = mybir.dt.float32
OP = mybir.AluOpType
AF = mybir.ActivationFunctionType

TAIL = 3.5
MINB = 0.01  # min bin width == min bin height
MIND = 0.01
EPS = 1e-6
NB = 8
SW = 2.0 * TAIL - MINB * NB  # 6.92 scale for widths and heights

N_TOTAL = 524288
N_CORES = 8
NC_ROWS = N_TOTAL // N_CORES  # 65536 rows per core
T_DEF = 128                   # rows per partition per tile
CBUFS_DEF = 1
SBUFS_DEF = 1
_FORCE_DVE = False            # debug: route gpsimd ops to DVE


def build_program(nc_rows: int = NC_ROWS, T: int = T_DEF,
                  cbufs: int = CBUFS_DEF, sbufs: int = SBUFS_DEF):
    """Build the single-core Bass program; rows = nc_rows are processed in
    tiles of 128*T rows."""
    P = 128
    R = P * T
    assert nc_rows % R == 0
    ntiles = nc_rows // R

    nc = bacc.Bacc("TRN2", target_bir_lowering=False, debug=False)

    p_dram = nc.dram_tensor("out_params", [nc_rows, 48], F32, kind="ExternalInput")
    e_dram = nc.dram_tensor("eps", [nc_rows, 2], F32, kind="ExternalInput")
    y_dram = nc.dram_tensor("y", [nc_rows, 2], F32, kind="ExternalOutput")

    kvec_dram = nc.inline_tensor(
        (np.arange(1, 8, dtype=np.float32) * MINB).reshape(7), name="kvec"
    )

    A = nc.scalar    # ACT engine
    V = nc.vector    # DVE
    GP = nc.vector if _FORCE_DVE else nc.gpsimd   # Pool/GpSimd
    SY = nc.sync     # SP (DMA dispatch)

    def stt(eng, out, in0, s, in1, op0, op1):
        return eng.scalar_tensor_tensor(
            out=out, in0=in0, scalar=float(s), in1=in1, op0=op0, op1=op1
        )

    def tt(eng, out, in0, in1, op):
        return eng.tensor_tensor(out=out, in0=in0, in1=in1, op=op)

    with tile.TileContext(nc) as tc, ExitStack() as ctx:
        io_pool = ctx.enter_context(tc.tile_pool(name="io", bufs=2))
        cpool = ctx.enter_context(tc.tile_pool(name="cpool", bufs=cbufs))
        spool = ctx.enter_context(tc.tile_pool(name="spool", bufs=sbufs))
        ones = ctx.enter_context(tc.tile_pool(name="ones", bufs=1))

        kvec = ones.tile([P, 7], F32)
        SY.dma_start(
            out=kvec,
            in_=bass.AP(tensor=kvec_dram, offset=0, ap=[[0, P], [1, 7]]),
        )

        for it in range(ntiles):
            r0 = it * R

            pt = io_pool.tile([P, T, 48], F32)
            et = io_pool.tile([P, T, 2], F32)
            SY.dma_start(
                out=pt,
                in_=p_dram[r0 : r0 + R, :].rearrange("(p t) c -> p t c", p=P),
            )
            SY.dma_start(
                out=et,
                in_=e_dram[r0 : r0 + R, :].rearrange("(p t) c -> p t c", p=P),
            )

            # --- transcendentals -------------------------------------------
            # G[:, :, g, b] = exp(raw) for groups g: (w_j0, w_j1, h_j0, h_j1)
            G = cpool.tile([P, T, 4, 8], F32)
            A.activation(
                out=G.rearrange("p t g b -> p t (g b)"),
                in_=pt[:, :, 2:34],
                func=AF.Exp,
            )
            # LnD[:, :, j, i] = softplus(raw_d) = ln(exp(raw_d) + 1)
            LnD = cpool.tile([P, T, 2, 7], F32)
            LnD_f = LnD.rearrange("p t j b -> p t (j b)")
            A.activation(out=LnD_f, in_=pt[:, :, 34:48], func=AF.Exp)
            A.activation(out=LnD_f, in_=LnD_f, func=AF.Ln, bias=1.0)

            # --- cumulative sums (exp space), S = C[..., 7] ----------------
            C = cpool.tile([P, T, 4, 8], F32)
            V.tensor_copy(out=C[:, :, :, 0], in_=G[:, :, :, 0])
            for k in range(1, 8):
                stt(V, C[:, :, :, k], G[:, :, :, k], 1.0, C[:, :, :, k - 1],
                    OP.mult, OP.add)

            # SW/S = exp(-ln S + ln SW) (off the critical path; only the
            # stage-C affine maps consume it)
            invS = cpool.tile([P, T, 4], F32)
            A.activation(out=invS, in_=C[:, :, :, 7], func=AF.Ln,
                         scale=1.0 / SW)
            A.activation(out=invS, in_=invS, func=AF.Exp, scale=-1.0)

            # --- inside mask and shifted x ---------------------------------
            M = spool.tile([P, T, 2], F32, tag="M")
            A.activation(out=M, in_=et, func=AF.Abs)
            V.tensor_scalar(out=M, in0=M, scalar1=TAIL, scalar2=None,
                            op0=OP.is_lt)
            XM35 = spool.tile([P, T, 2], F32, tag="XM35")
            stt(V, XM35, et, 1.0, M, OP.mult, OP.mult)        # x*m
            V.tensor_scalar(out=XM35, in0=XM35, scalar1=TAIL, scalar2=None,
                            op0=OP.add)                       # x*m + 3.5

            # --- searchsorted, unnormalized --------------------------------
            # ge_k = [ SW*C_w[k-1] + MINB*k*S_w <= S_w*xm35 ],  k = 1..7
            SX = cpool.tile([P, T, 2], F32)
            stt(V, SX, C[:, :, 0:2, 7], 1.0, XM35, OP.mult, OP.mult)
            KS = cpool.tile([P, T, 2, 7], F32)
            for j in range(2):
                ksj = KS[:, :, j]
                tt(V, ksj,
                   C[:, :, j, 7].unsqueeze(2).broadcast_to([P, T, 7]),
                   kvec.unsqueeze(1).broadcast_to([P, T, 7]), OP.mult)
                stt(V, ksj, C[:, :, j, 0:7], SW, ksj, OP.mult, OP.add)
                stt(V, ksj, ksj, 1.0,
                    SX[:, :, j].unsqueeze(2).broadcast_to([P, T, 7]),
                    OP.mult, OP.is_le)
            GE = KS

            # --- derivative delta arrays (bin-major; GP subgraph) ----------
            # derivs = [1, d_1..d_7, 1], d_i = MIND + LnD[i-1]
            # DD0 = diffs for d0-selection; the d1-selection diffs are the
            # same array shifted by one bin (rows 1..6 of DD0) plus a final
            # row D7 = 1 - d_7.
            DD0 = cpool.tile([P, T, 7, 2], F32)
            D7 = cpool.tile([P, T, 2], F32)
            for j in range(2):
                tt(GP, DD0[:, :, 1:7, j], LnD[:, :, j, 1:7],
                   LnD[:, :, j, 0:6], OP.subtract)
            V.tensor_scalar(out=DD0[:, :, 0, :], in0=LnD[:, :, :, 0],
                            scalar1=MIND - 1.0, scalar2=None, op0=OP.add)
            V.tensor_scalar(out=D7, in0=LnD[:, :, :, 6],
                            scalar1=-1.0, scalar2=1.0 - MIND,
                            op0=OP.mult, op1=OP.add)

            # --- masked sums -----------------------------------------------
            # A = sum_k ge_k * e_{k-1}   (exclusive-cumsum at bin)
            # B = sum_k ge_k * e_k       (e_sel = e_0 + B - A)
            # W-path (+ MINB*bin) on DVE; H-path + D-path on GP.
            SUMW = cpool.tile([P, T, 3, 2], F32)   # q: A_w, B_w, MINB*bin
            SUMH = cpool.tile([P, T, 4, 2], F32)   # q: A_h, B_h, Sdd0, Sdd1
            # view of G with groups split into (j, type): g = ty*2 + j
            G2 = G.rearrange("p t (ty j) b -> p t j ty b", ty=2)
            for j in range(2):
                gej = GE[:, :, j]
                PRW = cpool.tile([P, T, 7, 3], F32, name="PRW", tag="PRW",
                                 bufs=2 * cbufs)
                tt(V, PRW[:, :, :, 0], G2[:, :, j, 0, 0:7], gej, OP.mult)
                tt(V, PRW[:, :, :, 1], G2[:, :, j, 0, 1:8], gej, OP.mult)
                V.tensor_scalar(out=PRW[:, :, :, 2], in0=gej, scalar1=MINB,
                                scalar2=None, op0=OP.mult)
                tt(V, PRW[:, :, 0:3, :], PRW[:, :, 4:7, :],
                   PRW[:, :, 0:3, :], OP.add)
                tt(V, PRW[:, :, 0:2, :], PRW[:, :, 2:4, :],
                   PRW[:, :, 0:2, :], OP.add)
                tt(V, SUMW[:, :, :, j], PRW[:, :, 0, :], PRW[:, :, 1, :],
                   OP.add)

                PRH = cpool.tile([P, T, 7, 4], F32, name="PRH", tag="PRH",
                                 bufs=2 * cbufs)
                tt(GP, PRH[:, :, :, 0], G2[:, :, j, 1, 0:7], gej, OP.mult)
                tt(GP, PRH[:, :, :, 1], G2[:, :, j, 1, 1:8], gej, OP.mult)
                tt(GP, PRH[:, :, :, 2], DD0[:, :, :, j], gej, OP.mult)
                tt(GP, PRH[:, :, 0:6, 3], DD0[:, :, 1:7, j], gej[:, :, 0:6],
                   OP.mult)
                tt(GP, PRH[:, :, 6, 3], D7[:, :, j], gej[:, :, 6], OP.mult)
                tt(GP, PRH[:, :, 0:3, :], PRH[:, :, 4:7, :],
                   PRH[:, :, 0:3, :], OP.add)
                tt(V, PRH[:, :, 0:2, :], PRH[:, :, 2:4, :],
                   PRH[:, :, 0:2, :], OP.add)
                tt(V, SUMH[:, :, :, j], PRH[:, :, 0, :], PRH[:, :, 1, :],
                   OP.add)

            # --- selected quantities (joint over j) ------------------------
            A_w = SUMW[:, :, 0, :]
            B_w = SUMW[:, :, 1, :]
            MB = SUMW[:, :, 2, :]      # MINB * bin
            A_h = SUMH[:, :, 0, :]
            B_h = SUMH[:, :, 1, :]
            S0 = SUMH[:, :, 2, :]
            S1 = SUMH[:, :, 3, :]
            invSw = invS[:, :, 0:2]    # SW / S_w
            invSh = invS[:, :, 2:4]    # SW / S_h

            def tl(name, pool=spool):
                return pool.tile([P, T, 2], F32, name=name, tag=name)

            TH = tl("TH")     # theta' = xm35 - x0(+3.5)
            tt(V, TH, A_w, invSw, OP.mult)
            tt(V, TH, TH, MB, OP.add)
            tt(V, TH, XM35, TH, OP.subtract)

            WQ = tl("WQ")     # w' = w + EPS
            tt(V, WQ, B_w, A_w, OP.subtract)
            tt(V, WQ, WQ, G2[:, :, :, 0, 0], OP.add)
            tt(V, WQ, WQ, invSw, OP.mult)
            V.tensor_scalar(out=WQ, in0=WQ, scalar1=MINB + EPS, scalar2=None,
                            op0=OP.add)

            Y0 = tl("Y0")     # y0 in +3.5 space
            tt(V, Y0, A_h, invSh, OP.mult)
            tt(V, Y0, Y0, MB, OP.add)

            HH = tl("HH")     # h
            tt(V, HH, B_h, A_h, OP.subtract)
            tt(V, HH, HH, G2[:, :, :, 1, 0], OP.add)
            tt(V, HH, HH, invSh, OP.mult)
            V.tensor_scalar(out=HH, in0=HH, scalar1=MINB, scalar2=None,
                            op0=OP.add)

            D0 = tl("D0")
            V.tensor_scalar(out=D0, in0=S0, scalar1=1.0, scalar2=None,
                            op0=OP.add)
            D1 = tl("D1")
            stt(V, D1, S1, MIND, LnD[:, :, :, 0], OP.add, OP.add)

            # --- rational quadratic, scaled by w'^3 ------------------------
            #   u = th*om, v = u*w'
            #   numer = h*(h*th^2 + d0*v)
            #   denom = h*w'^2 + (d0+d1)*v - 2*h*u + EPS*w'^3
            OM = tl("OM")
            tt(V, OM, WQ, TH, OP.subtract)
            U = tl("U")
            tt(V, U, TH, OM, OP.mult)
            VV = tl("VV")
            tt(V, VV, U, WQ, OP.mult)

            # numerator branch (DVE)
            Q2 = tl("Q2")
            A.activation(out=Q2, in_=TH, func=AF.Square)
            NUM = tl("NUM")
            tt(V, NUM, Q2, HH, OP.mult)
            DVX = tl("DVX")
            tt(V, DVX, D0, VV, OP.mult)
            tt(V, NUM, NUM, DVX, OP.add)
            tt(V, NUM, NUM, HH, OP.mult)

            # denominator branch (GP)
            W2 = tl("W2")
            A.activation(out=W2, in_=WQ, func=AF.Square)
            DEN = tl("DEN")
            tt(GP, DEN, W2, HH, OP.mult)
            DS = tl("DS")
            tt(GP, DS, D0, D1, OP.add)
            tt(GP, DS, DS, VV, OP.mult)
            tt(GP, DEN, DEN, DS, OP.add)
            HU = tl("HU")
            tt(GP, HU, HH, U, OP.mult)
            tt(GP, HU, HU, HU, OP.add)                        # 2*h*u
            tt(GP, DEN, DEN, HU, OP.subtract)
            W3 = tl("W3")
            tt(GP, W3, W2, WQ, OP.mult)
            stt(V, DEN, W3, EPS, DEN, OP.mult, OP.add)

            # reciprocal of denom via exp(-ln(denom)) on ACT
            A.activation(out=DEN, in_=DEN, func=AF.Ln)
            A.activation(out=DEN, in_=DEN, func=AF.Exp, scale=-1.0)

            XYL = tl("XYL")
            tt(GP, XYL, et, pt[:, :, 0:2], OP.add)

            OUT = io_pool.tile([P, T, 2], F32)
            tt(V, OUT, NUM, DEN, OP.mult)                     # ratio
            YD = tl("YD")
            tt(V, YD, Y0, XM35, OP.subtract)                  # y0_35 - xm35
            tt(V, OUT, OUT, YD, OP.add)
            tt(V, OUT, OUT, M, OP.mult)
            tt(V, OUT, OUT, XYL, OP.add)

            SY.dma_start(
                out=y_dram[r0 : r0 + R, :].rearrange("(p t) c -> p t c", p=P),
                in_=OUT,
            )

    return nc


_CACHE = {}


def _get_program(nc_rows, T):
    key = (nc_rows, T)
    if key not in _CACHE:
        nc = build_program(nc_rows, T)
        nc.compile()
        _CACHE[key] = nc
    return _CACHE[key]


def kernel(out_params: np.ndarray, eps: np.ndarray) -> np.ndarray:
    assert out_params.shape == (N_TOTAL, 48), out_params.shape
    assert eps.shape == (N_TOTAL, 2), eps.shape
    out_params = np.ascontiguousarray(out_params, dtype=np.float32)
    eps = np.ascontiguousarray(eps, dtype=np.float32)

    nc = _get_program(NC_ROWS, T_DEF)
    core_ids = list(range(N_CORES))
    in_maps = [
        {
            "out_params": out_params[i * NC_ROWS : (i + 1) * NC_ROWS],
            "eps": eps[i * NC_ROWS : (i + 1) * NC_ROWS],
        }
        for i in core_ids
    ]
    res = run_bass_kernel_spmd(nc, in_maps, core_ids)
    return np.concatenate([r["y"] for r in res.results], axis=0)


# revision 27
# speedup vs baseline: 4682.0109x; 4682.0109x over previous
"""Trainium2 Bass kernel for the conditional-prior VAE rational-quadratic
spline loss head.

Math (per row n of out_params [N, 48], eps [N, 2], nb = 8):
  y_loc = out_params[:, :2]
  raw_w = out_params[:, 2:18]  -> per spline-subrow j: channels 2+8j .. 9+8j
  raw_h = out_params[:, 18:34] -> channels 18+8j .. 25+8j
  raw_d = out_params[:, 34:48] -> channels 34+7j .. 40+7j
  y[n, j] = y_loc[n, j] + RQspline(eps[n, j]; raw_w, raw_h, raw_d)

Kernel strategy: pure data parallel over 8 cores (batch split). Per core,
rows are laid out 128 per partition-dim x T per free-dim. All cross-bin
work (softmax, cumsum, searchsorted, gather) is done as elementwise ops
over per-bin channel slices of row-major tiles:
  - softmax/cumsum in exp-space; searchsorted runs unnormalized
    (SW*C_k + MINB*k*S <= S*(x+3.5)) so no reciprocal sits on the
    critical path; SW/S for the final affine maps comes from one ACT
    exp(-ln S + ln SW).
  - gather at bin = masked sums  sum_k ge_k * v_k  folded by a small
    adjacency tree
  - the rational quadratic is evaluated in a division-free form (numer
    and denom both scaled by (w+eps)^3) so a single reciprocal remains.
Engine split is by parallel subgraph: DVE owns cumsum/searchsorted/W-path/
stage-C/final; GpSimd (which only supports plain tensor_tensor/copy ops)
owns derivative deltas, H/D-path products, the first tree fold and the
denominator branch; ACT owns all transcendentals (one act-table).
"""

import sys

for _p in ("/opt/trn_rl_repo",):
    if _p not in sys.path:
        sys.path.insert(0, _p)

import math
from contextlib import ExitStack

import numpy as np

import concourse.bass as bass
import concourse.tile as tile
from concourse import bacc, mybir
from concourse.bass_utils import run_bass_kernel_spmd

# Steer every activation to the one table set that holds all functions this
# kernel uses (exp, ln, abs, square, copy, identity), so the schedule carries
# a single LoadActFuncSet instead of reloading on every exp<->ln switch.
# Index order of the table list is preserved (walrus remaps by index).
_AF = mybir.ActivationFunctionType
_KERNEL_FUNCS = {_AF.Exp, _AF.Ln, _AF.Abs, _AF.Square, _AF.Copy, _AF.Identity}
_ONE_TABLE = "natural_log_exp_and_others"
_orig_get_act_tables = bacc.get_activation_tables


def _patched_act_tables(arch):
    out = {}
    for name, funcs in _orig_get_act_tables(arch).items():
        if name == _ONE_TABLE:
            out[name] = funcs
        else:
            out[name] = funcs - _KERNEL_FUNCS
    return out


bacc.get_activation_tables = _patched_act_tables

F32 = mybir.dt.float32
OP = mybir.AluOpType
AF = mybir.ActivationFunctionType

TAIL = 3.5
MINB = 0.01  # min bin width == min bin height
MIND = 0.01
EPS = 1e-6
NB = 8
SW = 2.0 * TAIL - MINB * NB  # 6.92 scale for widths and heights

N_TOTAL = 524288
N_CORES = 8
NC_ROWS = N_TOTAL // N_CORES  # 65536 rows per core
T_DEF = 128                   # rows per partition per tile
CBUFS_DEF = 1
SBUFS_DEF = 1
_FORCE_DVE = False            # debug: route gpsimd ops to DVE


def build_program(nc_rows: int = NC_ROWS, T: int = T_DEF,
                  cbufs: int = CBUFS_DEF, sbufs: int = SBUFS_DEF,
                  repeat: int = 1):
    """Build the single-core Bass program; rows = nc_rows are processed in
    tiles of 128*T rows. repeat>1 wraps the whole pass in a hardware loop
    (benchmark use only: wall-time deltas isolate device time)."""
    P = 128
    R = P * T
    assert nc_rows % R == 0
    ntiles = nc_rows // R

    nc = bacc.Bacc("TRN2", target_bir_lowering=False, debug=False)

    p_dram = nc.dram_tensor("out_params", [nc_rows, 48], F32, kind="ExternalInput")
    e_dram = nc.dram_tensor("eps", [nc_rows, 2], F32, kind="ExternalInput")
    y_dram = nc.dram_tensor("y", [nc_rows, 2], F32, kind="ExternalOutput")

    kvec_dram = nc.inline_tensor(
        (np.arange(1, 8, dtype=np.float32) * MINB).reshape(7), name="kvec"
    )

    A = nc.scalar    # ACT engine
    V = nc.vector    # DVE
    GP = nc.vector if _FORCE_DVE else nc.gpsimd   # Pool/GpSimd
    SY = nc.sync     # SP (DMA dispatch)

    def stt(eng, out, in0, s, in1, op0, op1):
        return eng.scalar_tensor_tensor(
            out=out, in0=in0, scalar=float(s), in1=in1, op0=op0, op1=op1
        )

    def tt(eng, out, in0, in1, op):
        return eng.tensor_tensor(out=out, in0=in0, in1=in1, op=op)

    with tile.TileContext(nc) as tc, ExitStack() as ctx:
        io_pool = ctx.enter_context(tc.tile_pool(name="io", bufs=2))
        cpool = ctx.enter_context(tc.tile_pool(name="cpool", bufs=cbufs))
        spool = ctx.enter_context(tc.tile_pool(name="spool", bufs=sbufs))
        ones = ctx.enter_context(tc.tile_pool(name="ones", bufs=1))

        kvec = ones.tile([P, 7], F32)
        SY.dma_start(
            out=kvec,
            in_=bass.AP(tensor=kvec_dram, offset=0, ap=[[0, P], [1, 7]]),
        )

        def _tile_pass():
          for it in range(ntiles):
            r0 = it * R

            pt = io_pool.tile([P, T, 48], F32)
            et = io_pool.tile([P, T, 2], F32)
            SY.dma_start(
                out=pt,
                in_=p_dram[r0 : r0 + R, :].rearrange("(p t) c -> p t c", p=P),
            )
            SY.dma_start(
                out=et,
                in_=e_dram[r0 : r0 + R, :].rearrange("(p t) c -> p t c", p=P),
            )

            # --- transcendentals -------------------------------------------
            # G[:, :, g, b] = exp(raw) for groups g: (w_j0, w_j1, h_j0, h_j1)
            G = cpool.tile([P, T, 4, 8], F32)
            A.activation(
                out=G.rearrange("p t g b -> p t (g b)"),
                in_=pt[:, :, 2:34],
                func=AF.Exp,
            )
            # LnD[:, :, j, i] = softplus(raw_d) = ln(exp(raw_d) + 1)
            LnD = cpool.tile([P, T, 2, 7], F32)
            LnD_f = LnD.rearrange("p t j b -> p t (j b)")
            A.activation(out=LnD_f, in_=pt[:, :, 34:48], func=AF.Exp)
            A.activation(out=LnD_f, in_=LnD_f, func=AF.Ln, bias=1.0)

            # --- cumulative sums (exp space), S = C[..., 7] ----------------
            C = cpool.tile([P, T, 4, 8], F32)
            V.tensor_copy(out=C[:, :, :, 0], in_=G[:, :, :, 0])
            for k in range(1, 8):
                stt(V, C[:, :, :, k], G[:, :, :, k], 1.0, C[:, :, :, k - 1],
                    OP.mult, OP.add)

            # SW/S (off the critical path; only stage-C affines consume it)
            invS = cpool.tile([P, T, 4], F32)
            V.reciprocal_approx_fast(out=invS, in_=C[:, :, :, 7])
            V.tensor_scalar(out=invS, in0=invS, scalar1=SW, scalar2=None,
                            op0=OP.mult)

            # --- inside mask and shifted x ---------------------------------
            M = spool.tile([P, T, 2], F32, tag="M")
            A.activation(out=M, in_=et, func=AF.Abs)
            V.tensor_scalar(out=M, in0=M, scalar1=TAIL, scalar2=None,
                            op0=OP.is_lt)
            XM35 = spool.tile([P, T, 2], F32, tag="XM35")
            stt(V, XM35, et, 1.0, M, OP.mult, OP.mult)        # x*m
            V.tensor_scalar(out=XM35, in0=XM35, scalar1=TAIL, scalar2=None,
                            op0=OP.add)                       # x*m + 3.5

            # --- searchsorted, unnormalized --------------------------------
            # ge_k = [ SW*C_w[k-1] + MINB*k*S_w <= S_w*xm35 ],  k = 1..7
            SX = cpool.tile([P, T, 2], F32)
            stt(V, SX, C[:, :, 0:2, 7], 1.0, XM35, OP.mult, OP.mult)
            KS = cpool.tile([P, T, 2, 7], F32)
            for j in range(2):
                ksj = KS[:, :, j]
                tt(V, ksj,
                   C[:, :, j, 7].unsqueeze(2).broadcast_to([P, T, 7]),
                   kvec.unsqueeze(1).broadcast_to([P, T, 7]), OP.mult)
                stt(V, ksj, C[:, :, j, 0:7], SW, ksj, OP.mult, OP.add)
                stt(V, ksj, ksj, 1.0,
                    SX[:, :, j].unsqueeze(2).broadcast_to([P, T, 7]),
                    OP.mult, OP.is_le)
            GE = KS

            # --- derivative delta arrays (bin-major; GP subgraph) ----------
            # derivs = [1, d_1..d_7, 1], d_i = MIND + LnD[i-1]
            # DD0 = diffs for d0-selection; the d1-selection diffs are the
            # same array shifted by one bin (rows 1..6 of DD0) plus a final
            # row D7 = 1 - d_7.
            DD0 = cpool.tile([P, T, 7, 2], F32)
            D7 = cpool.tile([P, T, 2], F32)
            for j in range(2):
                tt(GP, DD0[:, :, 1:7, j], LnD[:, :, j, 1:7],
                   LnD[:, :, j, 0:6], OP.subtract)
            V.tensor_scalar(out=DD0[:, :, 0, :], in0=LnD[:, :, :, 0],
                            scalar1=MIND - 1.0, scalar2=None, op0=OP.add)
            V.tensor_scalar(out=D7, in0=LnD[:, :, :, 6],
                            scalar1=-1.0, scalar2=1.0 - MIND,
                            op0=OP.mult, op1=OP.add)

            # --- masked sums -----------------------------------------------
            # A = sum_k ge_k * e_{k-1}   (exclusive-cumsum at bin)
            # B = sum_k ge_k * e_k       (e_sel = e_0 + B - A)
            # W-path (+ MINB*bin) on DVE; H-path + D-path on GP.
            SUMW = cpool.tile([P, T, 3, 2], F32)   # q: A_w, B_w, MINB*bin
            SUMH = cpool.tile([P, T, 4, 2], F32)   # q: A_h, B_h, Sdd0, Sdd1
            # view of G with groups split into (j, type): g = ty*2 + j
            G2 = G.rearrange("p t (ty j) b -> p t j ty b", ty=2)
            for j in range(2):
                gej = GE[:, :, j]
                PRW = cpool.tile([P, T, 7, 3], F32, name="PRW", tag="PRW",
                                 bufs=2 * cbufs)
                tt(V, PRW[:, :, :, 0], G2[:, :, j, 0, 0:7], gej, OP.mult)
                tt(V, PRW[:, :, :, 1], G2[:, :, j, 0, 1:8], gej, OP.mult)
                V.tensor_scalar(out=PRW[:, :, :, 2], in0=gej, scalar1=MINB,
                                scalar2=None, op0=OP.mult)
                tt(V, PRW[:, :, 0:3, :], PRW[:, :, 4:7, :],
                   PRW[:, :, 0:3, :], OP.add)
                tt(V, PRW[:, :, 0:2, :], PRW[:, :, 2:4, :],
                   PRW[:, :, 0:2, :], OP.add)
                tt(V, SUMW[:, :, :, j], PRW[:, :, 0, :], PRW[:, :, 1, :],
                   OP.add)

                PRH = cpool.tile([P, T, 7, 4], F32, name="PRH", tag="PRH",
                                 bufs=2 * cbufs)
                tt(GP, PRH[:, :, :, 0], G2[:, :, j, 1, 0:7], gej, OP.mult)
                tt(GP, PRH[:, :, :, 1], G2[:, :, j, 1, 1:8], gej, OP.mult)
                tt(GP, PRH[:, :, :, 2], DD0[:, :, :, j], gej, OP.mult)
                tt(GP, PRH[:, :, 0:6, 3], DD0[:, :, 1:7, j], gej[:, :, 0:6],
                   OP.mult)
                tt(GP, PRH[:, :, 6, 3], D7[:, :, j], gej[:, :, 6], OP.mult)
                tt(GP, PRH[:, :, 0:3, :], PRH[:, :, 4:7, :],
                   PRH[:, :, 0:3, :], OP.add)
                tt(V, PRH[:, :, 0:2, :], PRH[:, :, 2:4, :],
                   PRH[:, :, 0:2, :], OP.add)
                tt(V, SUMH[:, :, :, j], PRH[:, :, 0, :], PRH[:, :, 1, :],
                   OP.add)

            # --- selected quantities (joint over j) ------------------------
            A_w = SUMW[:, :, 0, :]
            B_w = SUMW[:, :, 1, :]
            MB = SUMW[:, :, 2, :]      # MINB * bin
            A_h = SUMH[:, :, 0, :]
            B_h = SUMH[:, :, 1, :]
            S0 = SUMH[:, :, 2, :]
            S1 = SUMH[:, :, 3, :]
            invSw = invS[:, :, 0:2]    # SW / S_w
            invSh = invS[:, :, 2:4]    # SW / S_h

            def tl(name, pool=spool):
                return pool.tile([P, T, 2], F32, name=name, tag=name)

            TH = tl("TH")     # theta' = xm35 - x0(+3.5)
            tt(V, TH, A_w, invSw, OP.mult)
            tt(V, TH, TH, MB, OP.add)
            tt(V, TH, XM35, TH, OP.subtract)

            WQ = tl("WQ")     # w' = w + EPS
            tt(V, WQ, B_w, A_w, OP.subtract)
            tt(V, WQ, WQ, G2[:, :, :, 0, 0], OP.add)
            tt(V, WQ, WQ, invSw, OP.mult)
            V.tensor_scalar(out=WQ, in0=WQ, scalar1=MINB + EPS, scalar2=None,
                            op0=OP.add)

            Y0 = tl("Y0")     # y0 in +3.5 space
            tt(V, Y0, A_h, invSh, OP.mult)
            tt(V, Y0, Y0, MB, OP.add)

            HH = tl("HH")     # h
            tt(V, HH, B_h, A_h, OP.subtract)
            tt(V, HH, HH, G2[:, :, :, 1, 0], OP.add)
            tt(V, HH, HH, invSh, OP.mult)
            V.tensor_scalar(out=HH, in0=HH, scalar1=MINB, scalar2=None,
                            op0=OP.add)

            D0 = tl("D0")
            V.tensor_scalar(out=D0, in0=S0, scalar1=1.0, scalar2=None,
                            op0=OP.add)
            D1 = tl("D1")
            stt(V, D1, S1, MIND, LnD[:, :, :, 0], OP.add, OP.add)

            # --- rational quadratic, scaled by w'^3 ------------------------
            #   u = th*om, v = u*w'
            #   numer = h*(h*th^2 + d0*v)
            #   denom = h*w'^2 + (d0+d1)*v - 2*h*u + EPS*w'^3
            OM = tl("OM")
            tt(V, OM, WQ, TH, OP.subtract)
            U = tl("U")
            tt(V, U, TH, OM, OP.mult)
            VV = tl("VV")
            tt(V, VV, U, WQ, OP.mult)

            # numerator branch (DVE)
            Q2 = tl("Q2")
            A.activation(out=Q2, in_=TH, func=AF.Square)
            NUM = tl("NUM")
            tt(V, NUM, Q2, HH, OP.mult)
            DVX = tl("DVX")
            tt(V, DVX, D0, VV, OP.mult)
            tt(V, NUM, NUM, DVX, OP.add)
            tt(V, NUM, NUM, HH, OP.mult)

            # denominator branch (GP)
            W2 = tl("W2")
            A.activation(out=W2, in_=WQ, func=AF.Square)
            DEN = tl("DEN")
            tt(GP, DEN, W2, HH, OP.mult)
            DS = tl("DS")
            tt(GP, DS, D0, D1, OP.add)
            tt(GP, DS, DS, VV, OP.mult)
            tt(GP, DEN, DEN, DS, OP.add)
            HU = tl("HU")
            tt(GP, HU, HH, U, OP.mult)
            tt(GP, HU, HU, HU, OP.add)                        # 2*h*u
            tt(GP, DEN, DEN, HU, OP.subtract)
            W3 = tl("W3")
            tt(GP, W3, W2, WQ, OP.mult)
            stt(V, DEN, W3, EPS, DEN, OP.mult, OP.add)

            # reciprocal of denom (single custom DVE op, ~18 correct bits)
            RD = tl("RD")
            V.reciprocal_approx_fast(out=RD, in_=DEN)
            DEN = RD

            XYL = tl("XYL")
            tt(GP, XYL, et, pt[:, :, 0:2], OP.add)

            OUT = io_pool.tile([P, T, 2], F32)
            tt(V, OUT, NUM, DEN, OP.mult)                     # ratio
            YD = tl("YD")
            tt(V, YD, Y0, XM35, OP.subtract)                  # y0_35 - xm35
            tt(V, OUT, OUT, YD, OP.add)
            tt(V, OUT, OUT, M, OP.mult)
            tt(V, OUT, OUT, XYL, OP.add)

            SY.dma_start(
                out=y_dram[r0 : r0 + R, :].rearrange("(p t) c -> p t c", p=P),
                in_=OUT,
            )

        if repeat > 1:
            with tc.For_i(0, repeat, 1):
                _tile_pass()
        else:
            _tile_pass()

    return nc


_CACHE = {}


def _get_program(nc_rows, T):
    key = (nc_rows, T)
    if key not in _CACHE:
        nc = build_program(nc_rows, T)
        nc.compile()
        _CACHE[key] = nc
    return _CACHE[key]


def kernel(out_params: np.ndarray, eps: np.ndarray) -> np.ndarray:
    assert out_params.shape == (N_TOTAL, 48), out_params.shape
    assert eps.shape == (N_TOTAL, 2), eps.shape
    out_params = np.ascontiguousarray(out_params, dtype=np.float32)
    eps = np.ascontiguousarray(eps, dtype=np.float32)

    nc = _get_program(NC_ROWS, T_DEF)
    core_ids = list(range(N_CORES))
    in_maps = [
        {
            "out_params": out_params[i * NC_ROWS : (i + 1) * NC_ROWS],
            "eps": eps[i * NC_ROWS : (i + 1) * NC_ROWS],
        }
        for i in core_ids
    ]
    res = run_bass_kernel_spmd(nc, in_maps, core_ids)
    return np.concatenate([r["y"] for r in res.results], axis=0)
